# revision 2
# baseline (speedup 1.0000x reference)
"""MinGRU Trainium2 kernel.

Reference computation (per batch element b, sequence length T, hidden H):
    k  = x @ W_z + b_z                       # [T, H]
    th = x @ W_h + b_h                       # [T, H]
    a  = sigmoid(-k)            (= 1 - z)
    g  = where(th >= 0, th + 0.5, sigmoid(th)) == max(th + 0.5, sigmoid(th))
    b_ = sigmoid(k) * g         (= z * g)
    h[t] = a[t] * h[t-1] + b_[t]             # linear scan along T
Output h  # [B, T, H]

Strategy: data-parallel over batch (B=8 -> 8 NeuronCores). Host transposes
x[b] to [D, T] so both matmuls produce [H, T] tiles directly (contraction dim
D on partitions for both operands; W is already the lhsT layout [D, H]).
The recurrence runs on the Vector engine's TENSOR_TENSOR_SCAN along the free
(T) axis: state = (a * state) - t with t = (a-1)*g = -b_. Host transposes the
[H, T] result back to [T, H].
"""

import numpy as np

B, T, D, H = 8, 4096, 512, 512
N_CORES = 8
TCH = 512                 # PSUM chunk along T
NT = T // TCH             # 8
NM = H // 128             # 4 partition tiles of H
NK = D // 128             # 4 contraction tiles

_cache = {}


def _build():
    import concourse.tile as tile
    from concourse import bacc, mybir

    f32 = mybir.dt.float32
    f32r = mybir.dt.float32r
    AF = mybir.ActivationFunctionType
    ALU = mybir.AluOpType

    nc = bacc.Bacc("TRN2", target_bir_lowering=False, debug=False,
                   num_devices=N_CORES)

    xt_d = nc.dram_tensor("xt", [D, T], f32r, kind="ExternalInput").ap()
    wz_d = nc.dram_tensor("wz", [D, H], f32r, kind="ExternalInput").ap()
    wh_d = nc.dram_tensor("wh", [D, H], f32r, kind="ExternalInput").ap()
    nbz_d = nc.dram_tensor("nbz", [128, NM], f32, kind="ExternalInput").ap()
    bh_d = nc.dram_tensor("bh", [128, NM], f32, kind="ExternalInput").ap()
    bh5_d = nc.dram_tensor("bh5", [128, NM], f32, kind="ExternalInput").ap()
    ht_d = nc.dram_tensor("ht", [H, T], f32, kind="ExternalOutput").ap()

    with tile.TileContext(nc) as tc:
        with (
            tc.tile_pool(name="const", bufs=1) as const,
            tc.tile_pool(name="rows", bufs=2) as rows,
            tc.tile_pool(name="chunks", bufs=3) as chunks,
            tc.tile_pool(name="psum", bufs=4, space="PSUM") as psum,
        ):
            xt_s = [const.tile([128, T], f32r, tag=f"xt{k}", name=f"xt{k}") for k in range(NK)]
            for k in range(NK):
                nc.sync.dma_start(xt_s[k][:], xt_d[k * 128:(k + 1) * 128, :])
            wz_s = [const.tile([128, H], f32r, tag=f"wz{k}", name=f"wz{k}") for k in range(NK)]
            wh_s = [const.tile([128, H], f32r, tag=f"wh{k}", name=f"wh{k}") for k in range(NK)]
            for k in range(NK):
                nc.sync.dma_start(wz_s[k][:], wz_d[k * 128:(k + 1) * 128, :])
                nc.sync.dma_start(wh_s[k][:], wh_d[k * 128:(k + 1) * 128, :])
            nbz_s = const.tile([128, NM], f32, tag="nbz")
            nc.sync.dma_start(nbz_s[:], nbz_d[:])
            bh_s = const.tile([128, NM], f32, tag="bh")
            nc.sync.dma_start(bh_s[:], bh_d[:])
            bh5_s = const.tile([128, NM], f32, tag="bh5")
            nc.sync.dma_start(bh5_s[:], bh5_d[:])

            for m in range(NM):
                msl = slice(m * 128, (m + 1) * 128)
                a_row = rows.tile([128, T], f32, tag="a_row")
                t_row = rows.tile([128, T], f32, tag="t_row")
                for tch in range(NT):
                    tsl = slice(tch * TCH, (tch + 1) * TCH)
                    psK = psum.tile([128, TCH], f32, tag="psK")
                    for k in range(NK):
                        nc.tensor.matmul(psK[:], wz_s[k][:, msl],
                                         xt_s[k][:, tsl],
                                         start=(k == 0), stop=(k == NK - 1))
                    psT = psum.tile([128, TCH], f32, tag="psT")
                    for k in range(NK):
                        nc.tensor.matmul(psT[:], wh_s[k][:, msl],
                                         xt_s[k][:, tsl],
                                         start=(k == 0), stop=(k == NK - 1))
                    # a = sigmoid(-(k0 + b_z))
                    nc.scalar.activation(a_row[:, tsl], psK[:], AF.Sigmoid,
                                         bias=nbz_s[:, m:m + 1], scale=-1.0)
                    # sg = sigmoid(th0 + b_h)
                    sg = chunks.tile([128, TCH], f32, tag="sg")
                    nc.scalar.activation(sg[:], psT[:], AF.Sigmoid,
                                         bias=bh_s[:, m:m + 1], scale=1.0)
                    # u = max(th0 + (b_h + 0.5), sg)
                    u = chunks.tile([128, TCH], f32, tag="u")
                    nc.vector.scalar_tensor_tensor(
                        u[:], psT[:], bh5_s[:, m:m + 1], sg[:],
                        ALU.add, ALU.max)
                    # t = (a - 1) * u  (= -b_)
                    nc.vector.scalar_tensor_tensor(
                        t_row[:, tsl], a_row[:, tsl], 1.0, u[:],
                        ALU.subtract, ALU.mult)
                # h[t] = a[t]*h[t-1] - t[t]
                h_row = rows.tile([128, T], f32, tag="h_row")
                nc.vector.tensor_tensor_scan(h_row[:], a_row[:], t_row[:],
                                             0.0, ALU.mult, ALU.subtract)
                nc.sync.dma_start(ht_d[msl, :], h_row[:])

    nc.compile()
    return nc


def kernel(x, W_z, b_z, W_h, b_h):
    from concourse.bass_utils import run_bass_kernel_spmd

    if "nc" not in _cache:
        _cache["nc"] = _build()
    nc = _cache["nc"]

    x = np.asarray(x, dtype=np.float32)
    W_z = np.ascontiguousarray(np.asarray(W_z, dtype=np.float32))
    W_h = np.ascontiguousarray(np.asarray(W_h, dtype=np.float32))
    b_z = np.asarray(b_z, dtype=np.float32)
    b_h = np.asarray(b_h, dtype=np.float32)

    nbz = np.ascontiguousarray((-b_z).reshape(NM, 128).T)
    bh = np.ascontiguousarray(b_h.reshape(NM, 128).T)
    bh5 = np.ascontiguousarray((b_h + 0.5).reshape(NM, 128).T)

    in_maps = []
    for b in range(B):
        in_maps.append({
            "xt": np.ascontiguousarray(x[b].T),
            "wz": W_z,
            "wh": W_h,
            "nbz": nbz,
            "bh": bh,
            "bh5": bh5,
        })

    import os
    kwargs = {}
    if os.environ.get("KERNEL_TRACE"):
        kwargs = dict(trace=True, tmpdir=os.environ.get("KERNEL_TMPDIR"))
    res = run_bass_kernel_spmd(nc, in_maps, core_ids=list(range(N_CORES)),
                               **kwargs)
    _cache["last_results"] = res

    out = np.empty((B, T, H), dtype=np.float32)
    for b in range(B):
        out[b] = res.results[b]["ht"].T
    return out


# revision 3
# speedup vs baseline: 1.1427x; 1.1427x over previous
"""MinGRU Trainium2 kernel.

Reference computation (per batch element b, sequence length T, hidden H):
    k  = x @ W_z + b_z                       # [T, H]
    th = x @ W_h + b_h                       # [T, H]
    a  = sigmoid(-k)            (= 1 - z)
    g  = where(th >= 0, th + 0.5, sigmoid(th)) == max(th + 0.5, sigmoid(th))
    b_ = sigmoid(k) * g         (= z * g)
    h[t] = a[t] * h[t-1] + b_[t]             # linear scan along T
Output h  # [B, T, H]

Strategy: data-parallel over batch (B=8 -> 8 NeuronCores). Host transposes
x[b] to [D, T] so both matmuls produce [H, T] tiles directly (contraction dim
D on partitions for both operands; W is already the lhsT layout [D, H]).
Matmuls run in float32r (full-rate fp32 mode of the PE). The recurrence runs
on the Vector engine's TENSOR_TENSOR_SCAN along the free (T) axis:
state = (a * state) - t with t = (a-1)*g = -b_. Host transposes the [H, T]
result back to [T, H].
"""

import numpy as np

B, T, D, H = 8, 4096, 512, 512
N_CORES = 8
MMN = 512                 # matmul free dim (PSUM bank limit for fp32)
TCH = 1024                # elementwise / scan chunk along T
NT = T // TCH             # 4
NM = H // 128             # 4 partition tiles of H
NK = D // 128             # 4 contraction tiles

_cache = {}


def _build():
    import concourse.tile as tile
    from concourse import bacc, mybir

    f32 = mybir.dt.float32
    f32r = mybir.dt.float32r
    AF = mybir.ActivationFunctionType
    ALU = mybir.AluOpType

    nc = bacc.Bacc("TRN2", target_bir_lowering=False, debug=False,
                   num_devices=N_CORES)

    xt_d = nc.dram_tensor("xt", [D, T], f32r, kind="ExternalInput").ap()
    wz_d = nc.dram_tensor("wz", [D, H], f32r, kind="ExternalInput").ap()
    wh_d = nc.dram_tensor("wh", [D, H], f32r, kind="ExternalInput").ap()
    nbz_d = nc.dram_tensor("nbz", [128, NM], f32, kind="ExternalInput").ap()
    bh_d = nc.dram_tensor("bh", [128, NM], f32, kind="ExternalInput").ap()
    bh5_d = nc.dram_tensor("bh5", [128, NM], f32, kind="ExternalInput").ap()
    ht_d = nc.dram_tensor("ht", [H, T], f32, kind="ExternalOutput").ap()

    with tile.TileContext(nc) as tc:
        with (
            tc.tile_pool(name="const", bufs=1) as const,
            tc.tile_pool(name="chunks", bufs=4) as chunks,
            tc.tile_pool(name="psum", bufs=2, space="PSUM") as psum,
        ):
            # weights + biases first (small, needed by the first matmul)
            wz_s = [const.tile([128, H], f32r, tag=f"wz{k}", name=f"wz{k}")
                    for k in range(NK)]
            wh_s = [const.tile([128, H], f32r, tag=f"wh{k}", name=f"wh{k}")
                    for k in range(NK)]
            for k in range(NK):
                nc.sync.dma_start(wz_s[k][:], wz_d[k * 128:(k + 1) * 128, :])
                nc.sync.dma_start(wh_s[k][:], wh_d[k * 128:(k + 1) * 128, :])
            nbz_s = const.tile([128, NM], f32, tag="nbz")
            nc.sync.dma_start(nbz_s[:], nbz_d[:])
            bh_s = const.tile([128, NM], f32, tag="bh")
            nc.sync.dma_start(bh_s[:], bh_d[:])
            bh5_s = const.tile([128, NM], f32, tag="bh5")
            nc.sync.dma_start(bh5_s[:], bh5_d[:])

            # x[b].T streamed in T-chunks so compute starts after ~1 MB
            xt_s = [const.tile([128, T], f32r, tag=f"xt{k}", name=f"xt{k}")
                    for k in range(NK)]
            for tc_i in range(NT):
                tsl = slice(tc_i * TCH, (tc_i + 1) * TCH)
                for k in range(NK):
                    nc.sync.dma_start(xt_s[k][:, tsl], xt_d[k * 128:(k + 1) * 128, tsl])

            for m in range(NM):
                msl = slice(m * 128, (m + 1) * 128)
                h_prev = None
                for tc_i in range(NT):
                    tsl = slice(tc_i * TCH, (tc_i + 1) * TCH)
                    psK = psum.tile([128, TCH], f32, tag="psK")
                    psT = psum.tile([128, TCH], f32, tag="psT")
                    for half in range(TCH // MMN):
                        nsl = slice(tc_i * TCH + half * MMN,
                                    tc_i * TCH + (half + 1) * MMN)
                        osl = slice(half * MMN, (half + 1) * MMN)
                        for k in range(NK):
                            nc.tensor.matmul(psK[:, osl], wz_s[k][:, msl],
                                             xt_s[k][:, nsl],
                                             start=(k == 0), stop=(k == NK - 1))
                        for k in range(NK):
                            nc.tensor.matmul(psT[:, osl], wh_s[k][:, msl],
                                             xt_s[k][:, nsl],
                                             start=(k == 0), stop=(k == NK - 1))
                    # a = sigmoid(-(k0 + b_z))
                    a = chunks.tile([128, TCH], f32, tag="a")
                    nc.scalar.activation(a[:], psK[:], AF.Sigmoid,
                                         bias=nbz_s[:, m:m + 1], scale=-1.0)
                    # sg = sigmoid(th0 + b_h)
                    sg = chunks.tile([128, TCH], f32, tag="sg")
                    nc.scalar.activation(sg[:], psT[:], AF.Sigmoid,
                                         bias=bh_s[:, m:m + 1], scale=1.0)
                    # u = max(th0 + (b_h + 0.5), sg)
                    u = chunks.tile([128, TCH], f32, tag="u")
                    nc.vector.scalar_tensor_tensor(
                        u[:], psT[:], bh5_s[:, m:m + 1], sg[:],
                        ALU.add, ALU.max)
                    # t = (a - 1) * u  (= -b_)
                    tt = chunks.tile([128, TCH], f32, tag="tt")
                    nc.vector.scalar_tensor_tensor(
                        tt[:], a[:], 1.0, u[:], ALU.subtract, ALU.mult)
                    # h[t] = a[t]*h[t-1] - t[t]
                    h = chunks.tile([128, TCH], f32, tag="h")
                    init = 0.0 if h_prev is None else h_prev[:, TCH - 1:TCH]
                    nc.vector.tensor_tensor_scan(h[:], a[:], tt[:], init,
                                                 ALU.mult, ALU.subtract)
                    h_prev = h
                    nc.sync.dma_start(ht_d[msl, tsl], h[:])

    nc.compile()
    return nc


def kernel(x, W_z, b_z, W_h, b_h):
    from concourse.bass_utils import run_bass_kernel_spmd

    if "nc" not in _cache:
        _cache["nc"] = _build()
    nc = _cache["nc"]

    x = np.asarray(x, dtype=np.float32)
    W_z = np.ascontiguousarray(np.asarray(W_z, dtype=np.float32))
    W_h = np.ascontiguousarray(np.asarray(W_h, dtype=np.float32))
    b_z = np.asarray(b_z, dtype=np.float32)
    b_h = np.asarray(b_h, dtype=np.float32)

    nbz = np.ascontiguousarray((-b_z).reshape(NM, 128).T)
    bh = np.ascontiguousarray(b_h.reshape(NM, 128).T)
    bh5 = np.ascontiguousarray((b_h + 0.5).reshape(NM, 128).T)

    in_maps = []
    for b in range(B):
        in_maps.append({
            "xt": np.ascontiguousarray(x[b].T),
            "wz": W_z,
            "wh": W_h,
            "nbz": nbz,
            "bh": bh,
            "bh5": bh5,
        })

    import os
    kwargs = {}
    if os.environ.get("KERNEL_TRACE"):
        kwargs = dict(trace=True, tmpdir=os.environ.get("KERNEL_TMPDIR"))
    res = run_bass_kernel_spmd(nc, in_maps, core_ids=list(range(N_CORES)),
                               **kwargs)
    _cache["last_results"] = res

    out = np.empty((B, T, H), dtype=np.float32)
    for b in range(B):
        out[b] = res.results[b]["ht"].T
    return out
